# revision 103
# baseline (speedup 1.0000x reference)
"""Trainium2 Bass kernel for nn_NeuroManifoldBlock (dense transformer block with
FitzHugh-Nagumo-evolved attention scores), SPMD across 8 NeuronCores.

Sharding: cores 0-3 -> batch 0, cores 4-7 -> batch 1. Within a batch group of
4 cores: the sdr projection is feature-sharded and joined by a bf16 on-chip
AllGather; an AllToAll simultaneously redistributes the projection
feature-sharded -> token-sharded so each core gets its own 256 tokens
token-major without recomputing the projection; attention is head-sharded
(4 heads/core); MLP + output are token-sharded, fed by two pipelined bf16
ReduceScatters (token halves) that sum the per-head out-projection partials.
Collective groups: [[0-3],[4-7]].

Key algorithmic choices:
 - All matmuls in bf16 with fp32 PSUM accumulation.
 - The 4-step FHN IMEX integration is an elementwise function g(s) of the
   pre-softmax scores only (v0=s, w0=0); evaluated as a degree-10 polynomial
   fit of g on the reachable score range (|s| < 3.35): free ACT-Relu clamp on
   the PSUM->SBUF copy, ACT Square (quadratic init), then 3 custom-DVE
   Horner3 ops registered at runtime.
 - LayerNorm 1 folds into the QKV projection as a rank-1 matmul correction
   (K=1 accumulate of colsum(W) x mean-row) plus a per-token rsqrt scale.
 - Softmax without max-subtraction; the denominator comes free from a ones
   column interleaved into the V tiles (65-wide per head) so the ctx matmul
   also produces the per-query denominator row; causal masking is a bf16
   multiply with host-built tile masks on diagonal tiles only.
 - Weight/input DMA is consolidated into ~30 wide transfers (the HWDGE queue
   serializes ~625ns per DMA instruction, so many small loads dominate).
"""

import numpy as np
import ml_dtypes

from concourse import bass, bacc, tile
import concourse.mybir as mybir
from concourse.bass_utils import run_bass_kernel_spmd

# ---------------------------------------------------------------- constants
B, T, SDR, D, H, DH = 2, 1024, 2048, 1024, 16, 64
FFN = 2730
FFN_PAD = 2816          # 22 * 128
N_CORES = 8
GROUP = 4               # cores per batch
HPC = 4                 # heads per core
TPC = 256               # tokens per core
DT_, FA, FB, FTAU, FTH = 0.1, 0.7, 0.8, 12.5, 0.5
EPS = 1e-5
CLAMP = 3.35
POLY_DEG = 7

F32 = mybir.dt.float32
BF16 = mybir.dt.bfloat16
_bfd = ml_dtypes.bfloat16

VW = 260                # v tile width: 4 heads x (64 v cols + 1 ones col)
TRI = True              # causal span restriction in attention


def _bf16(x):
    return np.ascontiguousarray(np.asarray(x, np.float32).astype(_bfd))


def _f32(x):
    return np.ascontiguousarray(np.asarray(x, np.float32))


# ------------------------------------------------------- FHN poly (host fit)
def _fhn_g(s):
    s = np.asarray(s, np.float64)
    v = s.copy()
    w = np.zeros_like(s)
    wd = 1.0 + DT_ * FB / FTAU
    for _ in range(4):
        v = v + DT_ * (v - v ** 3 / 3.0 - w + s)
        w = (w + DT_ * (v + FA) / FTAU) / wd
    return v - FTH


def _fit_poly():
    # poly in u = relu(s_raw + 8*CLAMP), s_raw = unscaled scores (q.k)
    xs = np.linspace(0.0, 2 * 8 * CLAMP, 400001)
    g = _fhn_g(xs / 8.0 - CLAMP)
    c = np.polynomial.chebyshev.Chebyshev.fit(xs, g, POLY_DEG)
    return c.convert(kind=np.polynomial.Polynomial).coef[::-1].astype(np.float64)


POLY = _fit_poly()
_c0, _c1, _c2 = POLY[0], POLY[1], POLY[2]
SQ_SIGN = 1.0 if _c0 > 0 else -1.0
_SW = np.sqrt(abs(_c0))
SQ_SCALE = float(_SW)
SQ_BIAS = float(_SW * (_c1 / (2 * _c0)))
SQ_GAMMA = float(_c2 - _c1 ** 2 / (4 * _c0))
HC = [float(c) for c in POLY[3:]]
assert len(HC) == 5


# ------------------------------------------------- custom DVE ops (runtime)
def _register_custom_ops():
    from concourse import dve_ops as DO
    from concourse.dve_spec import Spec, Src0, Src1, C0, C1, C2, lower
    from concourse.dve_uop import DveOpSpec

    defs = {
        "ANT_TT_MULT_ADDC": Spec(
            body=Src0 * Src1 + C0,
            reference=lambda in0, in1, s0, s1, imm2: (
                in0.astype(np.float32) * in1 + s0),
        ),
        "ANT_TT_ADDC_MULT": Spec(
            body=(Src0 + C0) * Src1,
            reference=lambda in0, in1, s0, s1, imm2: (
                (in0.astype(np.float32) + s0) * in1),
        ),
        "ANT_MUL_C_ADD_T": Spec(
            body=Src0 * C0 + Src1,
            reference=lambda in0, in1, s0, s1, imm2: (
                in0.astype(np.float32) * s0 + in1),
        ),
        "ANT_H3_NEG": Spec(
            body=((C0 - Src0) * Src1 + C1) * Src1 + C2,
            reference=lambda in0, in1, s0, s1, imm2: (
                ((s0 - in0.astype(np.float32)) * in1 + s1) * in1 + imm2),
        ),
        "ANT_H3_POS": Spec(
            body=((C0 + Src0) * Src1 + C1) * Src1 + C2,
            reference=lambda in0, in1, s0, s1, imm2: (
                ((s0 + in0.astype(np.float32)) * in1 + s1) * in1 + imm2),
        ),
        "ANT_H3": Spec(
            body=((Src0 * Src1 + C0) * Src1 + C1) * Src1 + C2,
            reference=lambda in0, in1, s0, s1, imm2: (
                ((in0.astype(np.float32) * in1 + s0) * in1 + s1) * in1 + imm2),
        ),
        "ANT_H2": Spec(
            body=(Src0 * Src1 + C0) * Src1 + C1,
            reference=lambda in0, in1, s0, s1, imm2: (
                (in0.astype(np.float32) * in1 + s0) * in1 + s1),
        ),
    }
    existing = {op.name for op in DO.OPS}
    for name, spec in defs.items():
        if name in existing:
            continue
        row = max(DO._SUB_OPCODE_FOR_NAME.values()) + 1
        assert row < 0x20
        DO._SUB_OPCODE_FOR_NAME[name] = row
        shas = {}
        for ver in ("v3", "v4"):
            try:
                shas[ver] = DveOpSpec(
                    name=name, opcode=row, uops=lower(spec, ver=ver),
                    rd1_en=True).sha(ver)
            except Exception:
                pass
        op = DO.DveOp(name, spec, subdim=False, uops_sha=shas)
        DO.OPS.append(op)
        DO.CUSTOM_DVE_SPECS[name] = spec
    return {op.name: op for op in DO.OPS}


_OPS = _register_custom_ops()


# ----------------------------------------------------------- graph builder
def build_graph(debug=False, single=False):
    nc = bacc.Bacc("TRN2", target_bir_lowering=False, debug=False,
                   num_devices=(1 if single else N_CORES))

    # const APs for float biases used by non-Copy activations
    for val in (float(EPS), float(8.0 * CLAMP), float(SQ_BIAS)):
        if (F32, val) not in nc.const_aps.aps:
            t_ = nc.alloc_sbuf_tensor(
                f"const-f32-{abs(hash(val)) % 10**8}", [128, 1], F32)
            nc.gpsimd.memset(t_.ap(), val)
            nc.const_aps.aps[(F32, val)] = t_.ap()
    nc.all_engine_barrier()

    def din(name, shape, dtype):
        return nc.dram_tensor(name, list(shape), dtype, kind="ExternalInput").ap()

    # consolidated inputs (see _prep_in_maps for layouts)
    sdrT_pack = din("sdrT_pack", (128, 16 * 1024), BF16)
    wsdrmy_pack = din("wsdrmy_pack", (128, 16 * 256), BF16)
    wqk_pack = din("wqk_pack", (128, 8 * 512), BF16)
    wv_pack = din("wv_pack", (128, 8 * VW), BF16)
    wout_pack = din("wout_pack", (64, 4 * 1024), BF16)
    masks_pack = din("masks_pack", (128, 128), BF16)
    # eplace rows dt2*128+f: one-hot placing feature f of the core's slice
    # at its D-column (g*256 + dt2*128 + f)
    eplace = din("eplace", (2 * 128, 4 * 1024), BF16)
    identb_in = din("identb", (128, 128), BF16)
    identf_in = din("identf", (128, 128), F32)
    colpack = din("colpack", (128, 50), F32)
    rowpack = din("rowpack", (1, 512 + VW + 1024), F32)
    biasbc_in = din("bias_bc", (128, VW), F32)
    # wgu rows (fi, p): fi-block = per kk [gate 128 cols | up 128 cols]
    wgu = din("wgu", (22 * 128, 2048), BF16)
    wd_pack = din("wd_pack", (128, 22 * 1024), BF16)

    out_ap = nc.dram_tensor("out_slice", [TPC, D], F32,
                            kind="ExternalOutput").ap()
    dbg = {}
    if debug:
        def dout(name, shape, dtype=F32):
            dbg[name] = nc.dram_tensor(name, list(shape), dtype,
                                       kind="ExternalOutput").ap()
        dout("dbg_q", (4 * DH, T), BF16)
        dout("dbg_k", (4 * DH, T), BF16)
        dout("dbg_u00", (128, 2048))
        dout("dbg_h00", (128, 2048))
        dout("dbg_p00", (128, 2048), BF16)
        dout("dbg_rec0", (1, 512))
        dout("dbg_den0", (1, 512))
        dout("dbg_v", (T, VW), BF16)
        dout("dbg_ctx", (4 * DH, T), BF16)
        dout("dbg_h2", (TPC, D), BF16)
        dout("dbg_su", (FFN_PAD, TPC), BF16)

    TT = 2         # 512-token column tiles
    NDT = 8        # 128-feature tiles of D
    NKK = 16       # 128-row chunks of SDR

    from concourse.dve_ops import OPS as _ops_list
    OP = {o.name: o for o in _ops_list}
    H3S = OP["ANT_H3_POS"] if SQ_SIGN > 0 else OP["ANT_H3_NEG"]
    AF = mybir.ActivationFunctionType
    ALU = mybir.AluOpType
    RG = [[0, 1, 2, 3], [4, 5, 6, 7]]

    with tile.TileContext(nc) as tc:
        # alloc order defines the release stack (LIFO): longest-lived first
        pp = tc.alloc_tile_pool(name="persist", bufs=1)
        psp = tc.alloc_tile_pool(name="psum", bufs=1, space="PSUM")
        dram = tc.alloc_tile_pool(name="dram", bufs=1, space="DRAM")
        mwS = tc.alloc_tile_pool(name="mlpw", bufs=1)
        qkvp = tc.alloc_tile_pool(name="qkvp", bufs=1)
        ap_ = tc.alloc_tile_pool(name="attn", bufs=1)
        sp = tc.alloc_tile_pool(name="sdrp", bufs=1)

        # ---------------- persistent small tiles ----------------
        ones_col = pp.tile([128, 1], BF16, name="ones_col")
        nc.vector.memset(ones_col[:], 1.0)
        ones_row_f = pp.tile([1, 128], F32, name="ones_row_f")
        nc.vector.memset(ones_row_f[:], 1.0)
        ident_sb = pp.tile([128, 128], BF16, name="ident_sb")
        identf_sb = pp.tile([128, 128], F32, name="identf_sb")
        cp = pp.tile([128, 50], F32, name="colpack_sb")
        rp = pp.tile([1, 512 + VW + 1024], F32, name="rowpack_sb")
        biasbc = pp.tile([128, VW], F32, name="biasbc_sb")

        sdrb_my_t = [cp[:, i:i + 1] for i in range(2)]
        qkb_tiles = [cp[:, 2 + i:3 + i] for i in range(4)]
        gb_tiles = [cp[:, 6 + i:7 + i] for i in range(22)]
        ub_tiles = [cp[:, 28 + i:29 + i] for i in range(22)]
        qkcs_sb = rp[:, 0:512]
        vcs_sb = rp[:, 512:512 + VW]
        bout_sb = rp[:, 512 + VW:512 + VW + 1024]

        # head-pair tiles: partitions 0-63 = even head, 64-127 = odd head
        qhp = [qkvp.tile([128, T], BF16, name=f"qhp{i}", tag=f"qhp{i}")
               for i in range(2)]
        khp = [qkvp.tile([128, T], BF16, name=f"khp{i}", tag=f"khp{i}")
               for i in range(2)]
        vts = [qkvp.tile([128, VW], BF16, name=f"vts{i}", tag=f"vts{i}")
               for i in range(8)]
        # per-(head, token-half) ctx tiles so out-proj of one half never
        # serializes against the other half's attention writes
        ctx_sb = [[qkvp.tile([64, T // 2], BF16, name=f"ctx_sb{h}_{q}",
                             tag=f"ctx_sb{h}_{q}") for q in range(2)]
                  for h in range(HPC)]
        r_bcast = sp.tile([128, T], F32, name="r_bcast")
        negmu_row = sp.tile([1, T], F32, name="negmu_row")
        r_col = [sp.tile([128, 1], F32, name=f"r_col{i}", tag=f"r_col{i}")
                 for i in range(8)]

        # ---------------- phase 1: sdr projection ----------------
        # 4 chunks of 4 kk each so matmuls start when chunk 0 lands
        sdrT_c = []
        wsdrmy_c = []
        for j in range(4):
            st = sp.tile([128, 4 * 1024], BF16, name=f"sdrT_c{j}",
                         tag=f"sdrT_c{j}")
            nc.sync.dma_start(st[:], sdrT_pack[:, j * 4096:(j + 1) * 4096])
            sdrT_c.append(st)
            wt = sp.tile([128, 4 * 256], BF16, name=f"wsdrmy_c{j}",
                         tag=f"wsdrmy_c{j}")
            nc.sync.dma_start(wt[:], wsdrmy_pack[:, j * 1024:(j + 1) * 1024])
            wsdrmy_c.append(wt)
            if j == 0:
                # small constants after the critical first chunk pair
                nc.sync.dma_start(cp[:], colpack[:])
                nc.sync.dma_start(rp[:], rowpack[:])
                nc.sync.dma_start(biasbc[:], biasbc_in[:])
                nc.sync.dma_start(ident_sb[:], identb_in[:])
                nc.sync.dma_start(identf_sb[:], identf_in[:])

        ag_in = dram.tile([256, T], BF16, name="ag_in")
        ag_out = dram.tile([D, T], BF16, name="ag_out")

        # xout_big layout: [128, 2048], block (dt2) at cols dt2*1024 + tok
        # (lives until out-proj: its x-slice rides the ReduceScatter there)
        xout_big = qkvp.tile([128, 2048], BF16, name="xout_big")
        for dt2 in range(2):
            for tt_i in range(TT):
                ps = psp.tile([128, 512], F32, name="sdr_ps", tag="mm", bufs=3)
                for kk in range(NKK):
                    j, r = kk // 4, kk % 4
                    nc.tensor.matmul(
                        ps[:],
                        wsdrmy_c[j][:, r * 256 + dt2 * 128:
                                    r * 256 + (dt2 + 1) * 128],
                        sdrT_c[j][:, r * 1024 + tt_i * 512:
                                  r * 1024 + (tt_i + 1) * 512],
                        start=(kk == 0), stop=(kk == NKK - 1))
                nc.scalar.activation(
                    xout_big[:, dt2 * 1024 + tt_i * 512:
                             dt2 * 1024 + (tt_i + 1) * 512],
                    ps[:], AF.Identity, bias=sdrb_my_t[dt2])

        # ag_in write split per token half so it overlaps the dt2=1 compute
        for tt_i in range(2):
            nc.sync.dma_start(
                ag_in[:].rearrange("(d p) t -> p d t", d=2)[
                    :, :, tt_i * 512:(tt_i + 1) * 512],
                xout_big[:].rearrange("p (d t) -> p d t", d=2)[
                    :, :, tt_i * 512:(tt_i + 1) * 512])

        xall = sp.tile([128, NDT * 1024], BF16, name="xall")
        if single:
            # pipelined fake AllGather at half-column granularity: each copy
            # half feeds its xall slice while the next transfers
            for r in range(4):
                for hc in range(2):
                    nc.sync.dma_start(
                        ag_out[r * 256:(r + 1) * 256,
                               hc * 512:(hc + 1) * 512],
                        ag_in[:, hc * 512:(hc + 1) * 512])
                    nc.sync.dma_start(
                        xall[:].rearrange("p (d t) -> p d t", d=8)[
                            :, 2 * r:2 * r + 2,
                            hc * 512:(hc + 1) * 512],
                        ag_out[:].rearrange("(d p) t -> p d t", d=8)[
                            :, 2 * r:2 * r + 2,
                            hc * 512:(hc + 1) * 512])
        else:
            nc.gpsimd.collective_compute(
                "AllGather", mybir.AluOpType.bypass,
                ins=[ag_in.opt()], outs=[ag_out.opt()],
                replica_groups=RG)
            nc.sync.dma_start(
                xall[:].rearrange("p (d t) -> p d t", d=8),
                ag_out[:].rearrange("(d p) t -> p d t", d=8))
        x_bf = [xall[:, dd * 1024:(dd + 1) * 1024] for dd in range(NDT)]

        # LN1 stats from the gathered x
        mu_row = sp.tile([1, T], F32, name="mu_row")
        sxx_row = sp.tile([1, T], F32, name="sxx_row")
        for tt_i in range(TT):
            mu_ps = psp.tile([1, 512], F32, name="mu_ps", tag="acc", bufs=2)
            sxx_ps = psp.tile([1, 512], F32, name="sxx_ps", tag="acc", bufs=2)
            for dt_i in range(NDT):
                xsq = sp.tile([128, 512], BF16, name="xsq", tag="xsq", bufs=3)
                nc.vector.tensor_tensor(
                    xsq[:], x_bf[dt_i][:, tt_i * 512:(tt_i + 1) * 512],
                    x_bf[dt_i][:, tt_i * 512:(tt_i + 1) * 512], op=ALU.mult)
                nc.tensor.matmul(
                    mu_ps[:],
                    ones_col[:], x_bf[dt_i][:, tt_i * 512:(tt_i + 1) * 512],
                    start=(dt_i == 0), stop=(dt_i == NDT - 1))
                nc.tensor.matmul(
                    sxx_ps[:],
                    ones_col[:], xsq[:],
                    start=(dt_i == 0), stop=(dt_i == NDT - 1))
            nc.scalar.activation(mu_row[:, tt_i * 512:(tt_i + 1) * 512],
                                 mu_ps[:], AF.Copy, scale=1.0 / D)
            nc.scalar.activation(sxx_row[:, tt_i * 512:(tt_i + 1) * 512],
                                 sxx_ps[:], AF.Copy, scale=1.0 / D)

        # ---------------- LN1 stats finalize ----------------
        nc.vector.tensor_scalar(negmu_row[:], mu_row[:], -1.0, None,
                                op0=ALU.mult)
        musq = sp.tile([1, T], F32, name="musq", tag="rowtmp", bufs=2)
        nc.vector.tensor_tensor(musq[:], mu_row[:], mu_row[:], op=ALU.mult)
        var_row = sp.tile([1, T], F32, name="var_row", tag="rowtmp", bufs=2)
        nc.vector.tensor_tensor(var_row[:], sxx_row[:], musq[:],
                                op=ALU.subtract)
        lnv = sp.tile([1, T], F32, name="lnv", tag="rowtmp", bufs=2)
        nc.scalar.activation(lnv[:], var_row[:], AF.Ln, bias=EPS)
        r_row = sp.tile([1, T], F32, name="r_row", tag="rowtmp", bufs=2)
        nc.scalar.activation(r_row[:], lnv[:], AF.Exp, scale=-0.5)
        for tt_i in range(TT):
            rb_ps = psp.tile([128, 512], F32, name="rb_ps", tag="mm", bufs=3)
            nc.tensor.matmul(rb_ps[:], ones_row_f[:],
                             r_row[:, tt_i * 512:(tt_i + 1) * 512])
            nc.scalar.activation(r_bcast[:, tt_i * 512:(tt_i + 1) * 512],
                                 rb_ps[:], AF.Copy)
        for j in range(8):
            tp = psp.tile([128, 128], F32, name="tp", tag="quad", bufs=3)
            nc.tensor.transpose(tp[:], r_bcast[:, j * 128:(j + 1) * 128],
                                identf_sb[:])
            nc.vector.tensor_copy(r_col[j][:], tp[:, 0:1])

        # ---------------- phase 2: qkv ----------------
        wqk_sb = sp.tile([128, 8 * 512], BF16, name="wqk_sb")
        nc.sync.dma_start(wqk_sb[:], wqk_pack[:])
        wv_sb = sp.tile([128, 8 * VW], BF16, name="wv_sb")
        nc.sync.dma_start(wv_sb[:], wv_pack[:])

        for fp in range(4):
            for tt_i in range(TT):
                ps = psp.tile([128, 512], F32, name="qk_ps", tag="mm", bufs=3)
                for kk in range(NDT):
                    nc.tensor.matmul(
                        ps[:],
                        wqk_sb[:, kk * 512 + fp * 128:
                               kk * 512 + (fp + 1) * 128],
                        x_bf[kk][:, tt_i * 512:(tt_i + 1) * 512],
                        start=(kk == 0), stop=False)
                nc.tensor.matmul(
                    ps[:], qkcs_sb[:, fp * 128:(fp + 1) * 128],
                    negmu_row[:, tt_i * 512:(tt_i + 1) * 512],
                    start=False, stop=True)
                dst = (qhp if fp < 2 else khp)[fp % 2]
                nc.vector._custom_dve(
                    OP["ANT_TT_MULT_ADDC"],
                    out=dst[:, tt_i * 512:(tt_i + 1) * 512],
                    in0=ps[:],
                    in1=r_bcast[:, tt_i * 512:(tt_i + 1) * 512],
                    s0=qkb_tiles[fp])

        for vt in range(8):
            ps = psp.tile([128, VW], F32, name="v_ps", tag="mm", bufs=3)
            for kk in range(NDT):
                nc.tensor.matmul(
                    ps[:],
                    x_bf[kk][:, vt * 128:(vt + 1) * 128],
                    wv_sb[:, kk * VW:(kk + 1) * VW],
                    start=(kk == 0), stop=False)
            nc.tensor.matmul(
                ps[:], negmu_row[:, vt * 128:(vt + 1) * 128],
                vcs_sb[:], start=False, stop=True)
            # vts = ps * r + biasbc; ones cols: ps==0, biasbc==1 -> 1.0
            nc.vector._custom_dve(
                OP["ANT_MUL_C_ADD_T"], out=vts[vt][:], in0=ps[:],
                in1=biasbc[:], s0=r_col[vt][:])

        if debug:
            for i in range(2):
                nc.sync.dma_start(dbg["dbg_q"][i * 128:(i + 1) * 128, :],
                                  qhp[i][:])
                nc.sync.dma_start(dbg["dbg_k"][i * 128:(i + 1) * 128, :],
                                  khp[i][:])
            for vt in range(8):
                nc.sync.dma_start(dbg["dbg_v"][vt * 128:(vt + 1) * 128, :],
                                  vts[vt][:])

        sp.release()
        fhn = tc.alloc_tile_pool(name="fhn", bufs=1)

        # ---------------- phase 3: attention ----------------
        wout_sb = ap_.tile([64, 4 * 1024], BF16, name="wout_sb")
        nc.sync.dma_start(wout_sb[:], wout_pack[:])
        ep_sb = ap_.tile([128, 2 * 4096], BF16, name="ep_sb")
        nc.sync.dma_start(
            ep_sb[:].rearrange("p (d c) -> p d c", d=2),
            eplace[:].rearrange("(d p) c -> p d c", d=2))
        msk = fhn.tile([128, 128], BF16, name="msk")
        nc.sync.dma_start(msk[:], masks_pack[:])

        b_in = [dram.tile([T // 2, D], BF16, name=f"b_in{i}") for i in range(2)]
        b_out = [dram.tile([128, D], BF16, name=f"b_out{i}") for i in range(2)]

        def outproj_half(half):
            # out-projection for token half + its ReduceScatter; the core's
            # x slice rides the reduction via the eplace matmuls.
            # b_acc layout: [128, 4096], block (mt2) at cols mt2*1024 + dc*512
            b_acc = ap_.tile([128, 4096], BF16, name="b_acc", tag="b_acc",
                             bufs=2)
            for mt2 in range(4):
                mt = half * 4 + mt2
                for dc in range(2):
                    ps = psp.tile([128, 512], F32, name="op_ps", tag="mm",
                                  bufs=3)
                    for h in range(HPC):
                        nc.tensor.matmul(
                            ps[:],
                            ctx_sb[h][half][:, mt2 * 128:(mt2 + 1) * 128],
                            wout_sb[:, h * 1024 + dc * 512:
                                    h * 1024 + (dc + 1) * 512],
                            start=(h == 0), stop=False)
                    for dt2 in range(2):
                        nc.tensor.matmul(
                            ps[:],
                            xout_big[:, dt2 * 1024 + mt * 128:
                                     dt2 * 1024 + (mt + 1) * 128],
                            ep_sb[:, dt2 * 4096 + dc * 512:
                                  dt2 * 4096 + (dc + 1) * 512],
                            start=False, stop=False)
                    nc.tensor.matmul(
                        ps[:], ones_row_f[:],
                        bout_sb[:, dc * 512:(dc + 1) * 512],
                        start=False, stop=True)
                    nc.scalar.activation(
                        b_acc[:, mt2 * 1024 + dc * 512:
                              mt2 * 1024 + (dc + 1) * 512],
                        ps[:], AF.Copy)
            for mt2 in range(4):
                nc.sync.dma_start(
                    b_in[half][:].rearrange("(m p) (d c) -> p m d c",
                                            m=4, d=2)[:, mt2],
                    b_acc[:].rearrange("p (m d c) -> p m d c",
                                       m=4, d=2)[:, mt2])
            if single:
                nc.sync.dma_start(b_out[half][:], b_in[half][0:128, :])
            else:
                nc.gpsimd.collective_compute(
                    "ReduceScatter", mybir.AluOpType.add,
                    ins=[b_in[half].opt()], outs=[b_out[half].opt()],
                    replica_groups=RG)

        for h in range(HPC):
            for qt in range(TT):
                nkt = 4 * (qt + 1)
                n_mac = (nkt + 3) // 4
                ctx_ps = psp.tile([65, 512], F32, name="ctx_ps", tag="acc",
                                  bufs=2)
                for mac in range(n_mac):
                    kts = list(range(mac * 4, min((mac + 1) * 4, nkt)))
                    # causal span per kt: valid queries are >= kt*128
                    los = [max(0, kt * 128 - qt * 512) if TRI else 0
                           for kt in kts]
                    spans = [512 - lo for lo in los]
                    offs = list(np.cumsum([0] + spans[:-1]))
                    mwd = sum(spans)
                    u_buf = fhn.tile([128, 2048], F32, name="u_buf",
                                     tag="u_buf", bufs=3)
                    h_buf = fhn.tile([128, 2048], F32, name="h_buf",
                                     tag="h_buf", bufs=3)
                    p_buf = fhn.tile([128, 2048], BF16, name="p_buf",
                                     tag="p_buf", bufs=3)
                    hb = (h % 2) * 64
                    for i, kt in enumerate(kts):
                        lo, sw, off = los[i], spans[i], offs[i]
                        ps = psp.tile([128, 512], F32, name="s_ps", tag="mm",
                                      bufs=3)
                        nc.tensor.matmul(
                            ps[:, lo:512],
                            khp[h // 2][hb:hb + 64, kt * 128:(kt + 1) * 128],
                            qhp[h // 2][hb:hb + 64,
                                        qt * 512 + lo:(qt + 1) * 512])
                        if h == 0:
                            # head 0's relu on DVE to balance the ACT wall
                            nc.vector.tensor_scalar(
                                u_buf[:, off:off + sw], ps[:, lo:512],
                                8.0 * CLAMP, 0.0,
                                op0=ALU.add, op1=ALU.max)
                        else:
                            nc.scalar.activation(
                                u_buf[:, off:off + sw], ps[:, lo:512],
                                AF.Relu, bias=8.0 * CLAMP)
                    nc.scalar.activation(
                        h_buf[:, 0:mwd], u_buf[:, 0:mwd], AF.Square,
                        bias=SQ_BIAS, scale=SQ_SCALE)
                    nc.vector._custom_dve(
                        H3S, out=h_buf[:, 0:mwd], in0=h_buf[:, 0:mwd],
                        in1=u_buf[:, 0:mwd],
                        s0=SQ_GAMMA, s1=HC[0], imm2=HC[1])
                    nc.vector._custom_dve(
                        OP["ANT_H3"], out=h_buf[:, 0:mwd],
                        in0=h_buf[:, 0:mwd], in1=u_buf[:, 0:mwd],
                        s0=HC[2], s1=HC[3], imm2=HC[4])
                    nc.scalar.activation(p_buf[:, 0:mwd], h_buf[:, 0:mwd],
                                         AF.Exp)
                    for i, kt in enumerate(kts):
                        # diagonal 128-block needs the triangular mask
                        if kt * 128 >= qt * 512:
                            off = offs[i] + (0 if TRI else
                                             kt * 128 - qt * 512)
                            nc.gpsimd.tensor_tensor(
                                p_buf[:, off:off + 128],
                                p_buf[:, off:off + 128],
                                msk[:], op=ALU.mult)
                            if not TRI and off + 128 < offs[i] + 512:
                                nc.vector.memset(
                                    p_buf[:, off + 128:offs[i] + 512], 0.0)
                    if debug and h == 0 and qt == 0 and mac == 0:
                        nc.sync.dma_start(dbg["dbg_u00"], u_buf[:])
                        nc.sync.dma_start(dbg["dbg_h00"], h_buf[:])
                        nc.sync.dma_start(dbg["dbg_p00"], p_buf[:])
                    for i, kt in enumerate(kts):
                        lo, sw, off = los[i], spans[i], offs[i]
                        first = (mac == 0 and i == 0)
                        last = (mac == n_mac - 1) and (i == len(kts) - 1)
                        nc.tensor.matmul(
                            ctx_ps[:, lo:512],
                            vts[kt][:, h * 65:(h + 1) * 65],
                            p_buf[:, off:off + sw],
                            start=first, stop=last)
                # DVE ignores the partition base on PSUM reads, so the den
                # row (partition 64) must be evacuated by ACT first.
                den_sb = fhn.tile([1, 512], F32, name="den_sb", tag="den_sb",
                                  bufs=2)
                nc.scalar.activation(den_sb[:], ctx_ps[64:65, :], AF.Copy)
                rec_sb = fhn.tile([1, 512], F32, name="rec_sb", tag="rec_sb",
                                  bufs=2)
                nc.vector.reciprocal_approx_fast(rec_sb[:], den_sb[:])
                if debug and h == 0 and qt == 0:
                    nc.sync.dma_start(dbg["dbg_rec0"], rec_sb[:])
                    nc.sync.dma_start(dbg["dbg_den0"], den_sb[:])
                recb_ps = psp.tile([64, 512], F32, name="recb_ps", tag="quad",
                                   bufs=3)
                nc.tensor.matmul(recb_ps[:], ones_row_f[:, 0:64], rec_sb[:])
                recb_sb = fhn.tile([64, 512], F32, name="recb_sb",
                                   tag="recb_sb", bufs=2)
                nc.scalar.activation(recb_sb[:], recb_ps[:], AF.Copy)
                nc.vector.tensor_tensor(
                    ctx_sb[h][qt][:],
                    ctx_ps[0:64, :], recb_sb[:], op=ALU.mult)

        outproj_half(0)
        outproj_half(1)

        if debug:
            for h in range(HPC):
                for q in range(2):
                    nc.sync.dma_start(
                        dbg["dbg_ctx"][h * 64:(h + 1) * 64,
                                       q * 512:(q + 1) * 512],
                        ctx_sb[h][q][:])

        fhn.release()
        ap_.release()
        qkvp.release()
        mlp_pool = tc.alloc_tile_pool(name="mlp", bufs=1)

        # ---------------- phases 6-8: per token half, so half A's MLP runs
        # while half B's ReduceScatter is still in flight ----------------
        # b_out already contains x + attn_out (x rode the ReduceScatter)
        x2 = [mlp_pool.tile([128, D], BF16, name=f"x2_{i}", tag=f"x2_{i}")
              for i in range(2)]
        h2 = [mlp_pool.tile([128, D], BF16, name=f"h2_{i}", tag=f"h2_{i}")
              for i in range(2)]
        for tt_i in range(2):
            # issue from the ACT queue: the SP queue is blocked behind the
            # half-B b_in DMA at this point
            nc.scalar.dma_start(x2[tt_i][:], b_out[tt_i][:])

        def ln2_half(tt_i):
            sx = mlp_pool.tile([128, 1], F32, name="sx", tag="sx", bufs=2)
            nc.vector.reduce_sum(sx[:], x2[tt_i][:], axis=mybir.AxisListType.X)
            scratch = mlp_pool.tile([128, D], F32, name="scratch",
                                    tag="scratch", bufs=1)
            sxx2 = mlp_pool.tile([128, 1], F32, name="sxx2", tag="sxx2", bufs=2)
            nc.vector._custom_dve(
                OP["TENSOR_TENSOR_REDUCE"], out=scratch[:],
                in0=x2[tt_i][:], in1=x2[tt_i][:], s0=0.0, s1=1.0 / D,
                accum_out=sxx2[:])
            mu2 = mlp_pool.tile([128, 1], F32, name="mu2", tag="mu2", bufs=2)
            nc.vector.tensor_scalar(mu2[:], sx[:], 1.0 / D, None, op0=ALU.mult)
            mu2sq = mlp_pool.tile([128, 1], F32, name="mu2sq", tag="mu2sq",
                                  bufs=2)
            nc.vector.tensor_tensor(mu2sq[:], mu2[:], mu2[:], op=ALU.mult)
            var2 = mlp_pool.tile([128, 1], F32, name="var2", tag="var2", bufs=2)
            nc.vector.tensor_tensor(var2[:], sxx2[:], mu2sq[:],
                                    op=ALU.subtract)
            lnv2 = mlp_pool.tile([128, 1], F32, name="lnv2", tag="lnv2", bufs=2)
            nc.scalar.activation(lnv2[:], var2[:], AF.Ln, bias=EPS)
            r2 = mlp_pool.tile([128, 1], F32, name="r2", tag="r2", bufs=2)
            nc.scalar.activation(r2[:], lnv2[:], AF.Exp, scale=-0.5)
            nmr2 = mlp_pool.tile([128, 1], F32, name="nmr2", tag="nmr2", bufs=2)
            nc.vector.tensor_tensor(nmr2[:], mu2[:], r2[:], op=ALU.mult)
            nc.vector.tensor_scalar(nmr2[:], nmr2[:], -1.0, None, op0=ALU.mult)
            nc.vector.tensor_scalar(h2[tt_i][:], x2[tt_i][:], r2[:], nmr2[:],
                                    op0=ALU.mult, op1=ALU.add)
            # h2Tall layout: [128, 2048], col tt*1024 + dd*128
            for q4 in range(2):
                tpb = psp.tile([128, 512], BF16, name="tpb", tag="quad",
                               bufs=3)
                for i in range(4):
                    dd = q4 * 4 + i
                    nc.tensor.transpose(
                        tpb[:, i * 128:(i + 1) * 128],
                        h2[tt_i][:, dd * 128:(dd + 1) * 128], ident_sb[:])
                nc.vector.tensor_copy(
                    h2Tall[:, tt_i * 1024 + q4 * 512:
                           tt_i * 1024 + (q4 + 1) * 512], tpb[:])

        h2Tall = mlp_pool.tile([128, 2048], BF16, name="h2Tall")
        suTall = mlp_pool.tile([128, 22 * 256], BF16, name="suTall")

        # h2T moving view for fi matmuls: [128, (2 tt @1024), 128] per kk
        def h2T_mv(kk):
            return h2Tall[:].rearrange("p (t k c) -> p t k c", t=2, k=8)[
                :, :, kk, :]

        # ---------------- phase 7: gate/up (feature-major) ------------
        # weights stream fi-major: one [128, 2048] tile per fi
        # (per kk: [gate 128 cols | up 128 cols] at kk*256)
        def gateup_half(tt_i):
            if tt_i == 1:
                return
            for fi in range(22):
                wt = mwS.tile([128, 2048], BF16, name="wgu_s",
                              tag="wgu_s", bufs=4)
                nc.sync.dma_start(wt[:], wgu[fi * 128:(fi + 1) * 128, :])
                gps = psp.tile([128, TPC], F32, name="gps", tag="quad",
                               bufs=3)
                ups = psp.tile([128, TPC], F32, name="ups", tag="quad",
                               bufs=3)
                for kk in range(NDT):
                    nc.tensor.matmul(
                        gps[:], wt[:, kk * 256:kk * 256 + 128], h2T_mv(kk),
                        start=(kk == 0), stop=(kk == NDT - 1))
                    nc.tensor.matmul(
                        ups[:], wt[:, kk * 256 + 128:(kk + 1) * 256],
                        h2T_mv(kk),
                        start=(kk == 0), stop=(kk == NDT - 1))
                sil = mlp_pool.tile([128, TPC], BF16, name="sil", tag="sil",
                                    bufs=2)
                nc.scalar.activation(sil[:], gps[:], AF.Silu,
                                     bias=gb_tiles[fi])
                nc.vector._custom_dve(
                    OP["ANT_TT_ADDC_MULT"],
                    out=suTall[:, fi * 256:(fi + 1) * 256],
                    in0=ups[:], in1=sil[:], s0=ub_tiles[fi])

        # ---------------- phase 8: down + residual out, per half ----------
        out_sb = [mlp_pool.tile([128, D], F32, name=f"out_sb{i}",
                                tag=f"out_sb{i}") for i in range(2)]

        def down_half(tt_i):
            if tt_i == 1:
                return
            # wd streams kk-pair-major; 4 concurrent PSUM accumulators
            pss = [[psp.tile([128, 512], F32, name=f"dn_ps{dc}{tt}",
                             tag=("mm" if dc == 0 else "acc"),
                             bufs=(3 if dc == 0 else 2))
                    for tt in range(2)] for dc in range(2)]
            for kkp in range(11):
                wdt = mwS.tile([128, 2048], BF16, name="wd_s", tag="wd_s",
                               bufs=3)
                nc.sync.dma_start(
                    wdt[:], wd_pack[:, kkp * 2048:(kkp + 1) * 2048])
                for k2 in range(2):
                    kk = kkp * 2 + k2
                    for dc in range(2):
                        for tt in range(2):
                            nc.tensor.matmul(
                                pss[dc][tt][:],
                                suTall[:, kk * 256 + tt * 128:
                                       kk * 256 + (tt + 1) * 128],
                                wdt[:, k2 * 1024 + dc * 512:
                                    k2 * 1024 + (dc + 1) * 512],
                                start=(kk == 0), stop=(kk == 21))
            for dc in range(2):
                for tt in range(2):
                    nc.vector.tensor_tensor(
                        out_sb[tt][:, dc * 512:(dc + 1) * 512], pss[dc][tt][:],
                        x2[tt][:, dc * 512:(dc + 1) * 512], op=ALU.add)
            for tt in range(2):
                nc.sync.dma_start(out_ap[tt * 128:(tt + 1) * 128, :],
                                  out_sb[tt][:])

        ln2_half(0)
        ln2_half(1)
        gateup_half(0)
        down_half(0)

        if debug:
            for tt_i in range(2):
                nc.sync.dma_start(dbg["dbg_h2"][tt_i * 128:(tt_i + 1) * 128, :],
                                  h2[tt_i][:])
            for fi in range(22):
                nc.sync.dma_start(
                    dbg["dbg_su"][fi * 128:(fi + 1) * 128, 0:128],
                    suTall[:, fi * 256:fi * 256 + 128])
                nc.sync.dma_start(
                    dbg["dbg_su"][fi * 128:(fi + 1) * 128, 128:256],
                    suTall[:, fi * 256 + 128:fi * 256 + 256])

        mlp_pool.release()
        mwS.release()
        dram.release()
        psp.release()
        pp.release()

    nc.compile()
    return nc


# ------------------------------------------------------------- host prep
def _prep_in_maps(inputs):
    sdr = _f32(inputs["sdr"])
    sdr_w = _f32(inputs["sdr_w"])
    sdr_b = _f32(inputs["sdr_b"])
    w_qkv = _f32(inputs["w_qkv"])
    b_qkv = _f32(inputs["b_qkv"])
    w_out = _f32(inputs["w_out"])
    b_out = _f32(inputs["b_out"])
    ln1_g, ln1_b = _f32(inputs["ln1_g"]), _f32(inputs["ln1_b"])
    ln2_g, ln2_b = _f32(inputs["ln2_g"]), _f32(inputs["ln2_b"])
    w_gate, w_up, w_down = (_f32(inputs["w_gate"]), _f32(inputs["w_up"]),
                            _f32(inputs["w_down"]))

    wqkv_f = w_qkv * ln1_g[:, None]
    bqkv_f = ln1_b @ w_qkv + b_qkv
    wg_f = w_gate * ln2_g[:, None]
    bg_f = ln2_b @ w_gate
    wu_f = w_up * ln2_g[:, None]
    bu_f = ln2_b @ w_up

    wg_p = np.zeros((D, FFN_PAD), np.float32); wg_p[:, :FFN] = wg_f
    wu_p = np.zeros((D, FFN_PAD), np.float32); wu_p[:, :FFN] = wu_f
    wd_p = np.zeros((FFN_PAD, D), np.float32); wd_p[:FFN, :] = w_down
    gb_p = np.zeros((FFN_PAD,), np.float32); gb_p[:FFN] = bg_f
    ub_p = np.zeros((FFN_PAD,), np.float32); ub_p[:FFN] = bu_f

    # wgu rows (fi, p): per kk [gate 128 cols | up 128 cols] at kk*256
    # wg_p/wu_p are (D, FFN_PAD); block (fi, p=D-chunk-row kk) col kk*256
    wgu_h = np.zeros((22, 128, 2048), np.float32)
    for kk in range(8):
        wgu_h[:, :, kk * 256:kk * 256 + 128] = wg_p.reshape(
            8, 128, 22, 128)[kk].transpose(1, 0, 2)
        wgu_h[:, :, kk * 256 + 128:(kk + 1) * 256] = wu_p.reshape(
            8, 128, 22, 128)[kk].transpose(1, 0, 2)
    wgu = _bf16(wgu_h.reshape(22 * 128, 2048))
    # wd_pack: [128, 22*1024], chunk kk at cols kk*1024
    wd_pack = _bf16(np.ascontiguousarray(
        wd_p.reshape(22, 128, 1024).transpose(1, 0, 2)).reshape(128, 22 * 1024))

    jj = np.arange(128)[None, :]
    pp_ = np.arange(128)[:, None]
    # triangular mask for the diagonal 128-block of each k-tile
    masks_pack = _bf16((jj >= pp_).astype(np.float32))
    identb = _bf16(np.eye(128, dtype=np.float32))
    identf = _f32(np.eye(128, dtype=np.float32))

    # colpack: [128, 50] f32
    colpack = np.zeros((128, 50), np.float32)
    colpack[:, 6:28] = gb_p.reshape(22, 128).T
    colpack[:, 28:50] = ub_p.reshape(22, 128).T

    in_maps = []
    for c in range(N_CORES):
        b, g = c // GROUP, c % GROUP
        hs = slice(g * HPC * DH, (g * HPC + HPC) * DH)
        # sdrT_pack: [128, 16*1024], chunk kk at cols kk*1024
        sdrT_b = _bf16(sdr[b].T)
        sdrT_pack = np.ascontiguousarray(
            sdrT_b.reshape(16, 128, 1024).transpose(1, 0, 2)
        ).reshape(128, 16 * 1024)
        wsdr_my = _bf16(sdr_w[:, g * 256:(g + 1) * 256])
        wsdrmy_pack = np.ascontiguousarray(
            wsdr_my.reshape(16, 128, 256).transpose(1, 0, 2)
        ).reshape(128, 16 * 256)

        wq_s = wqkv_f[:, 0 * D:1 * D][:, hs]
        wk_s = wqkv_f[:, 1 * D:2 * D][:, hs]
        wv_s = wqkv_f[:, 2 * D:3 * D][:, hs]
        wqk_s = _bf16(np.concatenate([wq_s, wk_s], axis=1))
        wqk_pack = np.ascontiguousarray(
            wqk_s.reshape(8, 128, 512).transpose(1, 0, 2)
        ).reshape(128, 8 * 512)
        qk_b = np.concatenate([bqkv_f[0 * D:1 * D][hs], bqkv_f[1 * D:2 * D][hs]])
        qk_cs = wqk_s.astype(np.float32).sum(axis=0)[None, :]

        # wv widened to VW cols: head h at h*65..h*65+64, ones col zeroed
        wv_w = np.zeros((D, VW), np.float32)
        for h in range(HPC):
            wv_w[:, h * 65:h * 65 + 64] = wv_s[:, h * 64:(h + 1) * 64]
        wv_bf = _bf16(wv_w)
        wv_pack = np.ascontiguousarray(
            wv_bf.reshape(8, 128, VW).transpose(1, 0, 2)
        ).reshape(128, 8 * VW)
        v_cs = wv_bf.astype(np.float32).sum(axis=0)
        v_bias = bqkv_f[2 * D:3 * D][hs]
        # bias_bc: vbias in v cols, 1.0 in ones cols
        bias_row = np.zeros((VW,), np.float32)
        for h in range(HPC):
            bias_row[h * 65:h * 65 + 64] = v_bias[h * 64:(h + 1) * 64]
            bias_row[h * 65 + 64] = 1.0
        bias_bc = np.ascontiguousarray(
            np.tile(bias_row[None, :], (128, 1)).astype(np.float32))

        # wout_pack: [64, 4*1024], head h at cols h*1024
        wo = _bf16(w_out[hs, :])
        wout_pack = np.ascontiguousarray(
            wo.reshape(4, 64, 1024).transpose(1, 0, 2)).reshape(64, 4 * 1024)

        bout_row = (b_out if g == 0 else np.zeros_like(b_out))

        # placement matrix: feature f of slice-half dt2 -> D-col g*256+dt2*128+f
        epl = np.zeros((2 * 128, 4 * 1024), np.float32)
        for dt2 in range(2):
            for f in range(128):
                epl[dt2 * 128 + f, g * 256 + dt2 * 128 + f] = 1.0

        cpk = colpack.copy()
        cpk[:, 0:2] = sdr_b[g * 256:(g + 1) * 256].reshape(2, 128).T
        cpk[:, 2:6] = qk_b.reshape(4, 128).T

        rowpack = np.zeros((1, 512 + VW + 1024), np.float32)
        rowpack[0, 0:512] = qk_cs
        rowpack[0, 512:512 + VW] = v_cs
        rowpack[0, 512 + VW:] = bout_row

        in_maps.append({
            "sdrT_pack": sdrT_pack,
            "wsdrmy_pack": wsdrmy_pack,
            "wqk_pack": wqk_pack,
            "wv_pack": wv_pack,
            "wout_pack": wout_pack,
            "masks_pack": masks_pack,
            "eplace": _bf16(epl),
            "identb": identb,
            "identf": identf,
            "colpack": cpk,
            "rowpack": rowpack,
            "bias_bc": bias_bc,
            "wgu": wgu,
            "wd_pack": wd_pack,
        })
    return in_maps


_GRAPH_CACHE = {}


def _get_graph(debug=False):
    if debug not in _GRAPH_CACHE:
        _GRAPH_CACHE[debug] = build_graph(debug=debug)
    return _GRAPH_CACHE[debug]


def kernel(**inputs):
    nc = _get_graph(debug=False)
    in_maps = _prep_in_maps(inputs)
    res = run_bass_kernel_spmd(nc, in_maps, core_ids=list(range(N_CORES)))
    out = np.zeros((B, T, D), np.float32)
    for c in range(N_CORES):
        b, g = c // GROUP, c % GROUP
        sl = res.results[c]["out_slice"]
        out[b, g * 128:(g + 1) * 128, :] = sl[0:128]
        out[b, 512 + g * 128:512 + (g + 1) * 128, :] = sl[128:256]
    return out
